# revision 7
# baseline (speedup 1.0000x reference)
"""Distributed causal multi-head attention block (GPT-2 style) for 8 TRN2 NeuronCores.

Sharding: data-parallel over batch (4 groups of 2 cores) x tensor-parallel over
heads (2 groups of 8 heads). Core c handles batch c//2, head-group c%2.

Per-core pipeline (all matmuls bf16 with f32 PSUM accumulation):
  1. x -> bf16, transpose via DRAM bounce (DMA xbar transpose) -> xT [NX, S]
  2. qT,kT = (Wq|Wk)^T chunks @ xT   (feat-major, bias via ACT Identity)
     v = xT^T-chunks @ Wv            (S-major, bias via rank-1 matmul)
  3. per head: scores^T tiles = kT_h^T-slices @ qT_h (causally skipped),
     P^T = exp(scores/8) (+ triangular mask on diagonal blocks),
     a[q,65] = P^T-blocks^T @ [v_h | ones]  -> denominator in col 64,
     normalize rows by 1/denom -> a_loc bf16 [S, 512]
  4. pair AllGather of a_loc (1 MB bf16) -> full a for the batch [2S, 512]
  5. c_proj half-columns: out[q,512] = aT-chunks^T @ Wproj_half + bias
Host assembles out[b, :, hg*512:(hg+1)*512] from each core.
"""

import numpy as np

import concourse.bass as bass
import concourse.mybir as mybir
import concourse.tile as tile
from concourse import bacc
from concourse.bass_utils import run_bass_kernel_spmd
from concourse.masks import make_upper_triangular

F32 = mybir.dt.float32
BF16 = mybir.dt.bfloat16
AF = mybir.ActivationFunctionType
ALU = mybir.AluOpType

P = 128
S = 1024          # sequence length
NX = 1024         # model width
D = 64            # head dim
H_LOC = 8         # heads per core
FEAT = H_LOC * D  # 512 local attention features
NKC = NX // P     # 8 contraction chunks
NST = S // P      # 8 sequence tiles
VW = D + 1        # v block width incl. ones column (65)


def build():
    nc = bacc.Bacc(num_devices=8)
    x = nc.dram_tensor("x", [S, NX], F32, kind="ExternalInput")
    wqkv = nc.dram_tensor("wqkv", [NX, 3 * FEAT], F32, kind="ExternalInput")
    bqkv = nc.dram_tensor("bqkv", [3 * FEAT], F32, kind="ExternalInput")
    wproj = nc.dram_tensor("wproj", [NX, FEAT], F32, kind="ExternalInput")
    bproj = nc.dram_tensor("bproj", [FEAT], F32, kind="ExternalInput")
    out = nc.dram_tensor("out", [S, FEAT], F32, kind="ExternalOutput")

    with tile.TileContext(nc) as tc:
        with (
            tc.tile_pool(name="stage", bufs=3) as stage,       # f32 load staging
            tc.tile_pool(name="xcast", bufs=3) as xcast,       # bf16 cast staging
            tc.tile_pool(name="pt", bufs=16) as ptp,           # P^T blocks
            tc.tile_pool(name="small", bufs=8) as small,       # recip vectors
            tc.tile_pool(name="outp", bufs=3) as outp,         # out f32 tiles
            tc.tile_pool(name="ps_big", bufs=3, space="PSUM") as ps_big,
            tc.tile_pool(name="ps_sc", bufs=2, space="PSUM") as ps_sc,
            tc.tile_pool(name="ps_av", bufs=3, space="PSUM") as ps_av,
            tc.tile_pool(name="dram", bufs=1, space="DRAM") as dram,
            tc.tile_pool(name="resident", bufs=1) as res,
        ):
            # ---- resident SBUF tensors (distinct tags -> distinct slots) ----
            xT_all = res.tile([P, NKC * S], BF16, tag="xT_all")          # [NX, S] chunked
            wqkv_bf = res.tile([P, NKC * 3 * FEAT], BF16, tag="wqkv_bf")
            qkT_all = res.tile([P, 8 * S], BF16, tag="qkT_all")          # qT(0..3)|kT(4..7)
            v_sb = res.tile([P, NST * H_LOC * VW], BF16, tag="v_sb")
            a_sb = res.tile([P, NST * FEAT], BF16, tag="a_sb")
            wp_bf = res.tile([P, NKC * FEAT], BF16, tag="wp_bf")
            aT_all = res.tile([P, 16 * FEAT], BF16, tag="aT_all")
            bias_sb = res.tile([P, 8], F32, tag="bias_sb")
            bv_row = res.tile([1, FEAT], BF16, tag="bv_row")
            bp_row = res.tile([1, FEAT], BF16, tag="bp_row")
            ones_row = res.tile([1, P], BF16, tag="ones_row")
            utri = res.tile([P, P], BF16, tag="utri")

            nc.vector.memset(ones_row[:], 1.0)
            make_upper_triangular(nc, utri[:], val=1.0, diag=True)
            nc.vector.memset(v_sb[:], 1.0)

            # qkv bias columns 0..7 as [128,1] per feature tile (q: 0..3, k: 4..7)
            nc.sync.dma_start(bias_sb[:], bqkv[0:1024].rearrange("(t p) -> p t", p=P))
            bv_f = stage.tile([1, FEAT], F32, tag="rowstage")
            nc.sync.dma_start(bv_f[:], bqkv[1024:1536].rearrange("(a b) -> a b", a=1))
            nc.vector.tensor_copy(out=bv_row[:], in_=bv_f[:])
            bp_f = stage.tile([1, FEAT], F32, tag="rowstage")
            nc.sync.dma_start(bp_f[:], bproj.rearrange("(a b) -> a b", a=1))
            nc.vector.tensor_copy(out=bp_row[:], in_=bp_f[:])

            # ---- phase A: x -> bf16 -> DRAM bounce -> transposed xT ----
            x_bf = dram.tile([S, NX], BF16)
            for st in range(NST):
                xf = stage.tile([P, NX], F32, tag="xf")
                nc.sync.dma_start(xf[:], x[st * P : (st + 1) * P, :])
                xb = xcast.tile([P, NX], BF16, tag="xb")
                nc.vector.tensor_copy(out=xb[:], in_=xf[:])
                nc.sync.dma_start(x_bf[st * P : (st + 1) * P, :], xb[:])
            for kc in range(NKC):
                nc.sync.dma_start_transpose(
                    xT_all[:, kc * S : (kc + 1) * S],
                    x_bf[:, kc * P : (kc + 1) * P],
                )

            # ---- phase B: cast weights ----
            for kc in range(NKC):
                wf = stage.tile([P, 3 * FEAT], F32, tag="wf")
                nc.sync.dma_start(wf[:], wqkv[kc * P : (kc + 1) * P, :])
                nc.vector.tensor_copy(
                    out=wqkv_bf[:, kc * 3 * FEAT : (kc + 1) * 3 * FEAT], in_=wf[:]
                )
            for fc in range(NKC):
                wpf = stage.tile([P, FEAT], F32, tag="wpf")
                nc.sync.dma_start(wpf[:], wproj[fc * P : (fc + 1) * P, :])
                nc.vector.tensor_copy(out=wp_bf[:, fc * FEAT : (fc + 1) * FEAT], in_=wpf[:])

            # ---- phase C: qT, kT (feature-major) ----
            # wqkv cols: q = 0:512, k = 512:1024 -> feature tile ft covers cols ft*128
            for ft in range(8):
                for half in range(2):
                    ps = ps_big.tile([P, 512], F32)
                    for kc in range(NKC):
                        nc.tensor.matmul(
                            ps[:],
                            wqkv_bf[:, kc * 3 * FEAT + ft * P : kc * 3 * FEAT + (ft + 1) * P],
                            xT_all[:, kc * S + half * 512 : kc * S + (half + 1) * 512],
                            start=(kc == 0),
                            stop=(kc == NKC - 1),
                        )
                    nc.scalar.activation(
                        out=qkT_all[:, ft * S + half * 512 : ft * S + (half + 1) * 512],
                        in_=ps[:],
                        func=AF.Identity,
                        bias=bias_sb[:, ft : ft + 1],
                    )

            # ---- phase D: v (S-major) with interleaved ones columns ----
            for st in range(NST):
                ps = ps_big.tile([P, 512], F32)
                nc.tensor.matmul(
                    ps[:], ones_row[:, 0:P], bv_row[:], start=True, stop=False
                )
                for kc in range(NKC):
                    nc.tensor.matmul(
                        ps[:],
                        xT_all[:, kc * S + st * P : kc * S + (st + 1) * P],
                        wqkv_bf[:, kc * 3 * FEAT + 1024 : kc * 3 * FEAT + 1536],
                        start=False,
                        stop=(kc == NKC - 1),
                    )
                base = st * H_LOC * VW
                for h in range(H_LOC):
                    nc.vector.tensor_copy(
                        out=v_sb[:, base + h * VW : base + h * VW + D],
                        in_=ps[:, h * D : (h + 1) * D],
                    )

            # ---- phase E: attention (q-half outer, head inner, causal skip) ----
            for qh in range(2):
                for h in range(H_LOC):
                    prow = (h % 2) * D
                    qcol = (h // 2) * S            # qT feature-tile col base
                    kcol = (4 + h // 2) * S        # kT feature-tile col base
                    pt_blocks = []
                    for j in range(4 * qh + 4):
                        dloc = j - 4 * qh          # diagonal block index in this half
                        coff = max(dloc, 0) * P    # first allowed local q col
                        ncols = 512 - coff
                        ps = ps_sc.tile([P, 512], F32)
                        ptb = ptp.tile([P, 512], BF16, tag="pt")
                        nc.tensor.matmul(
                            ps[:, coff:512],
                            qkT_all[prow : prow + D, kcol + j * P : kcol + (j + 1) * P],
                            qkT_all[
                                prow : prow + D,
                                qcol + qh * 512 + coff : qcol + (qh + 1) * 512,
                            ],
                            start=True,
                            stop=True,
                        )
                        nc.scalar.activation(
                            out=ptb[:, coff:512],
                            in_=ps[:, coff:512],
                            func=AF.Exp,
                            scale=0.125,
                        )
                        if dloc >= 0:
                            nc.vector.tensor_tensor(
                                out=ptb[:, coff : coff + P],
                                in0=ptb[:, coff : coff + P],
                                in1=utri[:],
                                op=ALU.mult,
                            )
                        pt_blocks.append(ptb)
                    for lt in range(4):
                        t = 4 * qh + lt
                        psa = ps_av.tile([P, VW], F32)
                        for j in range(t + 1):
                            nc.tensor.matmul(
                                psa[:],
                                pt_blocks[j][:, lt * P : (lt + 1) * P],
                                v_sb[:, j * H_LOC * VW + h * VW : j * H_LOC * VW + (h + 1) * VW],
                                start=(j == 0),
                                stop=(j == t),
                            )
                        rc = small.tile([P, 1], F32, tag="rc")
                        nc.vector.reciprocal(rc[:], psa[:, D : D + 1])
                        nc.vector.tensor_scalar_mul(
                            out=a_sb[:, t * FEAT + h * D : t * FEAT + (h + 1) * D],
                            in0=psa[:, 0:D],
                            scalar1=rc[:],
                        )

            # ---- phase F: pair AllGather of a_loc ----
            cc_in = dram.tile([S, FEAT], BF16)
            cc_out = dram.tile([2 * S, FEAT], BF16)
            for st in range(NST):
                nc.sync.dma_start(
                    cc_in[st * P : (st + 1) * P, :], a_sb[:, st * FEAT : (st + 1) * FEAT]
                )
            nc.gpsimd.collective_compute(
                "AllGather",
                ALU.bypass,
                replica_groups=[[0, 1], [2, 3], [4, 5], [6, 7]],
                ins=[cc_in[:].opt()],
                outs=[cc_out[:].opt()],
            )

            # ---- phase G: transposed reload of gathered a ----
            # gathered feature chunk fc (0..7): rank block fc//4, cols (fc%4)*128
            for fc in range(NKC):
                for qh2 in range(2):
                    nc.sync.dma_start_transpose(
                        aT_all[:, (fc * 2 + qh2) * FEAT : (fc * 2 + qh2 + 1) * FEAT],
                        cc_out[
                            (fc // 4) * S + qh2 * 512 : (fc // 4) * S + (qh2 + 1) * 512,
                            (fc % 4) * P : (fc % 4 + 1) * P,
                        ],
                    )

            # ---- phase H: c_proj half-columns ----
            for t in range(NST):
                ps = ps_big.tile([P, 512], F32)
                nc.tensor.matmul(
                    ps[:], ones_row[:, 0:P], bp_row[:], start=True, stop=False
                )
                for fc in range(NKC):
                    nc.tensor.matmul(
                        ps[:],
                        aT_all[
                            :,
                            (fc * 2 + t // 4) * FEAT + (t % 4) * P
                            : (fc * 2 + t // 4) * FEAT + (t % 4 + 1) * P,
                        ],
                        wp_bf[:, fc * FEAT : (fc + 1) * FEAT],
                        start=False,
                        stop=(fc == NKC - 1),
                    )
                ot = outp.tile([P, FEAT], F32, tag="ot")
                nc.scalar.copy(ot[:], ps[:])
                nc.sync.dma_start(out[t * P : (t + 1) * P, :], ot[:])

    nc.finalize()
    return nc


_NC_CACHE = None
_LAST_IN_MAPS = None


def kernel(x, c_attn_w, c_attn_b, c_proj_w, c_proj_b):
    global _NC_CACHE, _LAST_IN_MAPS
    x = np.asarray(x, dtype=np.float32)
    c_attn_w = np.asarray(c_attn_w, dtype=np.float32)
    c_attn_b = np.asarray(c_attn_b, dtype=np.float32)
    c_proj_w = np.asarray(c_proj_w, dtype=np.float32)
    c_proj_b = np.asarray(c_proj_b, dtype=np.float32)
    B = x.shape[0]
    assert x.shape == (B, S, NX)

    in_maps = []
    for c in range(8):
        b, hg = c // 2, c % 2
        cols = slice(hg * FEAT, (hg + 1) * FEAT)
        wq = c_attn_w[:, 0 * NX :][:, cols]
        wk = c_attn_w[:, 1 * NX :][:, cols]
        wv = c_attn_w[:, 2 * NX :][:, cols]
        bq = c_attn_b[0 * NX :][cols]
        bk = c_attn_b[1 * NX :][cols]
        bv = c_attn_b[2 * NX :][cols]
        in_maps.append(
            {
                "x": np.ascontiguousarray(x[b]),
                "wqkv": np.ascontiguousarray(np.concatenate([wq, wk, wv], axis=1)),
                "bqkv": np.ascontiguousarray(np.concatenate([bq, bk, bv])),
                "wproj": np.ascontiguousarray(c_proj_w[:, cols]),
                "bproj": np.ascontiguousarray(c_proj_b[cols]),
            }
        )

    _LAST_IN_MAPS = in_maps
    if _NC_CACHE is None:
        _NC_CACHE = build()
    res = run_bass_kernel_spmd(_NC_CACHE, in_maps, core_ids=list(range(8)))
    outf = np.empty((B, S, NX), dtype=np.float32)
    for c in range(8):
        b, hg = c // 2, c % 2
        outf[b, :, hg * FEAT : (hg + 1) * FEAT] = res.results[c]["out"]
    return outf


# revision 11
# speedup vs baseline: 1.0248x; 1.0248x over previous
"""Distributed causal multi-head attention block (GPT-2 style) for 8 TRN2 NeuronCores.

Sharding: data-parallel over batch (4 groups of 2 cores) x tensor-parallel over
heads (2 groups of 8 heads). Core c handles batch c//2, head-group c%2.

Per-core pipeline (all matmuls bf16 with f32 PSUM accumulation):
  1. x -> bf16, transpose via DRAM bounce (DMA xbar transpose) -> xT [NX, S]
  2. qT,kT = (Wq|Wk)^T chunks @ xT   (feat-major, bias via ACT Identity)
     v = xT^T-chunks @ Wv            (S-major, bias via rank-1 matmul)
  3. per head: scores^T tiles = kT_h^T-slices @ qT_h (causally skipped),
     P^T = exp(scores/8) (+ triangular mask on diagonal blocks),
     a[q,65] = P^T-blocks^T @ [v_h | ones]  -> denominator in col 64,
     normalize rows by 1/denom -> a_loc bf16 [S, 512]
  4. pair AllGather of a_loc (1 MB bf16) -> full a for the batch [2S, 512]
  5. c_proj half-columns: out[q,512] = aT-chunks^T @ Wproj_half + bias
Host assembles out[b, :, hg*512:(hg+1)*512] from each core.
"""

import numpy as np

import concourse.bass as bass
import concourse.mybir as mybir
import concourse.tile as tile
from concourse import bacc
from concourse.bass_utils import run_bass_kernel_spmd
from concourse.masks import make_upper_triangular

F32 = mybir.dt.float32
BF16 = mybir.dt.bfloat16
AF = mybir.ActivationFunctionType
ALU = mybir.AluOpType

P = 128
S = 1024          # sequence length
NX = 1024         # model width
D = 64            # head dim
H_LOC = 8         # heads per core
FEAT = H_LOC * D  # 512 local attention features
NKC = NX // P     # 8 contraction chunks
NST = S // P      # 8 sequence tiles
VW = D + 1        # v block width incl. ones column (65)


def build():
    nc = bacc.Bacc(num_devices=8)
    x = nc.dram_tensor("x", [S, NX], F32, kind="ExternalInput")
    wqkv = nc.dram_tensor("wqkv", [NX, 3 * FEAT], F32, kind="ExternalInput")
    bqkv = nc.dram_tensor("bqkv", [3 * FEAT], F32, kind="ExternalInput")
    wproj = nc.dram_tensor("wproj", [NX, FEAT], F32, kind="ExternalInput")
    bproj = nc.dram_tensor("bproj", [FEAT], F32, kind="ExternalInput")
    out = nc.dram_tensor("out", [S, FEAT], F32, kind="ExternalOutput")

    with tile.TileContext(nc) as tc:
        with (
            tc.tile_pool(name="stage", bufs=3) as stage,       # f32 load staging
            tc.tile_pool(name="xcast", bufs=3) as xcast,       # bf16 cast staging
            tc.tile_pool(name="pt", bufs=16) as ptp,           # P^T blocks
            tc.tile_pool(name="small", bufs=8) as small,       # recip vectors
            tc.tile_pool(name="outp", bufs=3) as outp,         # out f32 tiles
            tc.tile_pool(name="ps_big", bufs=3, space="PSUM") as ps_big,
            tc.tile_pool(name="ps_sc", bufs=3, space="PSUM") as ps_sc,
            tc.tile_pool(name="ps_av", bufs=2, space="PSUM") as ps_av,
            tc.tile_pool(name="dram", bufs=1, space="DRAM") as dram,
            tc.tile_pool(name="resident", bufs=1) as res,
        ):
            # ---- resident SBUF tensors (distinct tags -> distinct slots) ----
            xT_all = res.tile([P, NKC * S], BF16, tag="xT_all")          # [NX, S] chunked
            wqkv_bf = res.tile([P, NKC * 3 * FEAT], BF16, tag="wqkv_bf")
            qkT_all = res.tile([P, 8 * S], BF16, tag="qkT_all")          # qT(0..3)|kT(4..7)
            v_sb = res.tile([P, NST * H_LOC * VW], BF16, tag="v_sb")
            a_sb = res.tile([P, NST * FEAT], BF16, tag="a_sb")
            wp_bf = res.tile([P, NKC * FEAT], BF16, tag="wp_bf")
            aT_all = res.tile([P, 16 * FEAT], BF16, tag="aT_all")
            bias_sb = res.tile([P, 8], F32, tag="bias_sb")
            bv_row = res.tile([1, FEAT], BF16, tag="bv_row")
            bp_row = res.tile([1, FEAT], BF16, tag="bp_row")
            ones_row = res.tile([1, P], BF16, tag="ones_row")
            utri = res.tile([P, P], BF16, tag="utri")

            nc.vector.memset(ones_row[:], 1.0)
            make_upper_triangular(nc, utri[:], val=1.0, diag=True)
            nc.vector.memset(v_sb[:], 1.0)

            # qkv bias columns 0..7 as [128,1] per feature tile (q: 0..3, k: 4..7)
            nc.sync.dma_start(bias_sb[:], bqkv[0:1024].rearrange("(t p) -> p t", p=P))
            bv_f = stage.tile([1, FEAT], F32, tag="rowstage")
            nc.sync.dma_start(bv_f[:], bqkv[1024:1536].rearrange("(a b) -> a b", a=1))
            nc.vector.tensor_copy(out=bv_row[:], in_=bv_f[:])
            bp_f = stage.tile([1, FEAT], F32, tag="rowstage")
            nc.sync.dma_start(bp_f[:], bproj.rearrange("(a b) -> a b", a=1))
            nc.vector.tensor_copy(out=bp_row[:], in_=bp_f[:])

            # ---- phase A: x -> bf16 -> DRAM bounce -> transposed xT ----
            # column-slab order so each NX-chunk's transpose fires as soon as
            # its slab is cast (pipeline startup instead of full-x barrier)
            x_bf = dram.tile([NKC * S, P], BF16)  # slab kc at rows kc*S
            for kc in range(NKC):
                xf = stage.tile([P, NST, P], F32, tag="xf")
                nc.sync.dma_start(
                    xf[:],
                    x[:, kc * P : (kc + 1) * P].rearrange("(st p) c -> p st c", p=P),
                )
                xb = xcast.tile([P, NST, P], BF16, tag="xb")
                nc.vector.tensor_copy(out=xb[:], in_=xf[:])
                nc.sync.dma_start(
                    x_bf[kc * S : (kc + 1) * S, :].rearrange("(st p) c -> p st c", p=P),
                    xb[:],
                )
                nc.sync.dma_start_transpose(
                    xT_all[:, kc * S : (kc + 1) * S],
                    x_bf[kc * S : (kc + 1) * S, :],
                )

            # ---- phase B: cast weights ----
            for kc in range(NKC):
                wf = stage.tile([P, 3 * FEAT], F32, tag="wf")
                nc.sync.dma_start(wf[:], wqkv[kc * P : (kc + 1) * P, :])
                nc.vector.tensor_copy(
                    out=wqkv_bf[:, kc * 3 * FEAT : (kc + 1) * 3 * FEAT], in_=wf[:]
                )
            for fc in range(NKC):
                wpf = stage.tile([P, FEAT], F32, tag="wpf")
                nc.sync.dma_start(wpf[:], wproj[fc * P : (fc + 1) * P, :])
                nc.vector.tensor_copy(out=wp_bf[:, fc * FEAT : (fc + 1) * FEAT], in_=wpf[:])

            # ---- phase C: qT, kT (feature-major) ----
            # wqkv cols: q = 0:512, k = 512:1024 -> feature tile ft covers cols ft*128
            for ft in range(8):
                for half in range(2):
                    ps = ps_big.tile([P, 512], F32)
                    for kc in range(NKC):
                        nc.tensor.matmul(
                            ps[:],
                            wqkv_bf[:, kc * 3 * FEAT + ft * P : kc * 3 * FEAT + (ft + 1) * P],
                            xT_all[:, kc * S + half * 512 : kc * S + (half + 1) * 512],
                            start=(kc == 0),
                            stop=(kc == NKC - 1),
                        )
                    nc.scalar.activation(
                        out=qkT_all[:, ft * S + half * 512 : ft * S + (half + 1) * 512],
                        in_=ps[:],
                        func=AF.Identity,
                        bias=bias_sb[:, ft : ft + 1],
                    )

            # ---- phase D: v (S-major) with interleaved ones columns ----
            for st in range(NST):
                ps = ps_big.tile([P, 512], F32)
                nc.tensor.matmul(
                    ps[:], ones_row[:, 0:P], bv_row[:], start=True, stop=False
                )
                for kc in range(NKC):
                    nc.tensor.matmul(
                        ps[:],
                        xT_all[:, kc * S + st * P : kc * S + (st + 1) * P],
                        wqkv_bf[:, kc * 3 * FEAT + 1024 : kc * 3 * FEAT + 1536],
                        start=False,
                        stop=(kc == NKC - 1),
                    )
                base = st * H_LOC * VW
                for h in range(H_LOC):
                    nc.vector.tensor_copy(
                        out=v_sb[:, base + h * VW : base + h * VW + D],
                        in_=ps[:, h * D : (h + 1) * D],
                    )

            # ---- phases E-H: attention, chunked AllGather, c_proj ----
            # Per q-half: attention -> bounce -> AllGather of that half.
            # c_proj for half 0 overlaps half-1 attention and AllGather #1.
            cc_in = dram.tile([S, FEAT], BF16)
            cc_outs = [dram.tile([2 * 512, FEAT], BF16, name=f"cc_out{i}") for i in range(2)]

            def attention_half(qh):
                for h in range(H_LOC):
                    prow = (h % 2) * D
                    qcol = (h // 2) * S            # qT feature-tile col base
                    kcol = (4 + h // 2) * S        # kT feature-tile col base
                    pt_blocks = []
                    for j in range(4 * qh + 4):
                        dloc = j - 4 * qh          # diagonal block index in this half
                        coff = max(dloc, 0) * P    # first allowed local q col
                        ncols = 512 - coff
                        ps = ps_sc.tile([P, 512], F32)
                        ptb = ptp.tile([P, 512], BF16, tag="pt")
                        nc.tensor.matmul(
                            ps[:, coff:512],
                            qkT_all[prow : prow + D, kcol + j * P : kcol + (j + 1) * P],
                            qkT_all[
                                prow : prow + D,
                                qcol + qh * 512 + coff : qcol + (qh + 1) * 512,
                            ],
                            start=True,
                            stop=True,
                        )
                        nc.scalar.activation(
                            out=ptb[:, coff:512],
                            in_=ps[:, coff:512],
                            func=AF.Exp,
                            scale=0.125,
                        )
                        if dloc >= 0:
                            nc.vector.tensor_tensor(
                                out=ptb[:, coff : coff + P],
                                in0=ptb[:, coff : coff + P],
                                in1=utri[:],
                                op=ALU.mult,
                            )
                        pt_blocks.append(ptb)
                    for lt in range(4):
                        t = 4 * qh + lt
                        psa = ps_av.tile([P, VW], F32)
                        for j in range(t + 1):
                            nc.tensor.matmul(
                                psa[:],
                                pt_blocks[j][:, lt * P : (lt + 1) * P],
                                v_sb[:, j * H_LOC * VW + h * VW : j * H_LOC * VW + (h + 1) * VW],
                                start=(j == 0),
                                stop=(j == t),
                            )
                        rc = small.tile([P, 1], F32, tag="rc")
                        nc.vector.reciprocal(rc[:], psa[:, D : D + 1])
                        nc.vector.tensor_scalar_mul(
                            out=a_sb[:, t * FEAT + h * D : t * FEAT + (h + 1) * D],
                            in0=psa[:, 0:D],
                            scalar1=rc[:],
                        )

            def gather_half(qh):
                # bounce this half's a_loc rows and AllGather within the pair
                for lt in range(4):
                    t = 4 * qh + lt
                    nc.sync.dma_start(
                        cc_in[t * P : (t + 1) * P, :],
                        a_sb[:, t * FEAT : (t + 1) * FEAT],
                    )
                nc.gpsimd.collective_compute(
                    "AllGather",
                    ALU.bypass,
                    replica_groups=[[0, 1], [2, 3], [4, 5], [6, 7]],
                    ins=[cc_in[qh * 512 : (qh + 1) * 512, :].opt()],
                    outs=[cc_outs[qh][:].opt()],
                )

            def proj_half(qh2):
                # transposed reload of gathered half, then c_proj for its rows
                # gathered chunk fc (0..7): rank block fc//4, cols (fc%4)*128
                for fc in range(NKC):
                    nc.scalar.dma_start_transpose(
                        aT_all[:, (fc * 2 + qh2) * FEAT : (fc * 2 + qh2 + 1) * FEAT],
                        cc_outs[qh2][
                            (fc // 4) * 512 : (fc // 4 + 1) * 512,
                            (fc % 4) * P : (fc % 4 + 1) * P,
                        ],
                    )
                for lt in range(4):
                    t = 4 * qh2 + lt
                    ps = ps_big.tile([P, 512], F32)
                    nc.tensor.matmul(
                        ps[:], ones_row[:, 0:P], bp_row[:], start=True, stop=False
                    )
                    for fc in range(NKC):
                        nc.tensor.matmul(
                            ps[:],
                            aT_all[
                                :,
                                (fc * 2 + qh2) * FEAT + lt * P
                                : (fc * 2 + qh2) * FEAT + (lt + 1) * P,
                            ],
                            wp_bf[:, fc * FEAT : (fc + 1) * FEAT],
                            start=False,
                            stop=(fc == NKC - 1),
                        )
                    ot = outp.tile([P, FEAT], F32, tag="ot")
                    nc.scalar.copy(ot[:], ps[:])
                    nc.sync.dma_start(out[t * P : (t + 1) * P, :], ot[:])

            attention_half(0)
            gather_half(0)
            attention_half(1)
            gather_half(1)
            proj_half(0)  # depends only on AllGather #0 -> overlaps AG #1
            proj_half(1)

    nc.finalize()
    return nc


_NC_CACHE = None
_LAST_IN_MAPS = None


def kernel(x, c_attn_w, c_attn_b, c_proj_w, c_proj_b):
    global _NC_CACHE, _LAST_IN_MAPS
    x = np.asarray(x, dtype=np.float32)
    c_attn_w = np.asarray(c_attn_w, dtype=np.float32)
    c_attn_b = np.asarray(c_attn_b, dtype=np.float32)
    c_proj_w = np.asarray(c_proj_w, dtype=np.float32)
    c_proj_b = np.asarray(c_proj_b, dtype=np.float32)
    B = x.shape[0]
    assert x.shape == (B, S, NX)

    in_maps = []
    for c in range(8):
        b, hg = c // 2, c % 2
        cols = slice(hg * FEAT, (hg + 1) * FEAT)
        wq = c_attn_w[:, 0 * NX :][:, cols]
        wk = c_attn_w[:, 1 * NX :][:, cols]
        wv = c_attn_w[:, 2 * NX :][:, cols]
        bq = c_attn_b[0 * NX :][cols]
        bk = c_attn_b[1 * NX :][cols]
        bv = c_attn_b[2 * NX :][cols]
        in_maps.append(
            {
                "x": np.ascontiguousarray(x[b]),
                "wqkv": np.ascontiguousarray(np.concatenate([wq, wk, wv], axis=1)),
                "bqkv": np.ascontiguousarray(np.concatenate([bq, bk, bv])),
                "wproj": np.ascontiguousarray(c_proj_w[:, cols]),
                "bproj": np.ascontiguousarray(c_proj_b[cols]),
            }
        )

    _LAST_IN_MAPS = in_maps
    if _NC_CACHE is None:
        _NC_CACHE = build()
    res = run_bass_kernel_spmd(_NC_CACHE, in_maps, core_ids=list(range(8)))
    outf = np.empty((B, S, NX), dtype=np.float32)
    for c in range(8):
        b, hg = c // 2, c % 2
        outf[b, :, hg * FEAT : (hg + 1) * FEAT] = res.results[c]["out"]
    return outf
